# revision 27
# baseline (speedup 1.0000x reference)
"""Trainium2 Bass kernel for patch attention:
    out = softmax(silu(q) @ silu(k)^T * scale, axis=-1)
with q,k: [B=4, H=16, P=1024, D=128] fp32, scale: [1] fp32.

Sharding: B*H = 64 heads split across 8 NeuronCores, 8 heads each.

v6 design.  bf16 on the wire both ways (host casts inputs to bf16, upcasts
the bf16 output back to fp32); qT/kT loaded via xbar dma_start_transpose
(no PE transposes).  Per-head engine budget (us), measured rates:

  ACT   exp 3 pairs [128,2048] + 2 singles w/ accum + tanh = 10.6  <- clock
  DVE   silu stt 2.2 + 3 pair row-sum reduces (1x) 6.9     =  9.1
  Pool  8x normalize_recip [128,1024] f32->bf16            =  8.6
  PE    16 matmul 128x128x512 bf16 (+8 ldweights)          ~  8
  DMA   0.5 MB in + 2 MB out at ~400 GB/s                  ~  6.3

Design notes, from HW traces + the cost model:
* Row sums are the scarce resource: every DVE reduction path measures 1x
  (~1.1ns/elem) — tensor_reduce, pool, AND tensor_scalar-with-accum_out
  (the cost model advertises 2x_2p for the latter but silicon runs 1x).
  So tiles 0-5 batch exp in [128,2048] pairs (amortizing ACT's ~350-cycle
  per-op overhead; accum_out there would mix the two tiles' sums) with
  one DVE pair-reduce each, and tiles 6-7 run unbatched exp WITH
  accum_out, making their sums nearly free on ACT (+180ns read_accum).
* exp pairs read 4-bank PSUM tiles; 2 buffers = all 8 banks.  The
  exp(p) -> matmul(p+2) -> exp(p+2) handoff is latency-tight; a burst of
  dummy matmuls during the DMA ramp warms the PE p-state (cold PE runs
  MM 128x128x512 at ~390ns vs ~216ns spec) to soften it.
* Strided/3D DVE access patterns cost ~2.6x, so everything DVE touches is
  contiguous; score m-tiles stay in natural row order and the output DMA
  writes 2 KB per (partition, row) chunk instead (256 descs/start).
* tanh shares the exp ACT table set (one ACT_TABLE_LOAD); head 0's tanh
  is split in half so ACT starts on qT while kT is still in flight.
  silu(g+1) is emitted after head g's reduces so its stt never
  head-of-line blocks DVE.  The scale load is queued on sync BEFORE the
  transposes (it gates the first exp).
* Normalization runs on Pool in steady state: concurrent DVE
  tensor_scalar + Pool normalize_recip contend on SBUF ports (a 0.6us DVE
  mult measured 5us when overlapped with Pool normalize).  Only the last
  head's odd tiles normalize on DVE, and its output DMAs are split per
  tile-pair, so the drain runs Pool/DVE in parallel.
* Issuing the k-half dma_start_transpose from nc.scalar races (local
  head 1 wrong on 7 of 8 cores) — all input DMAs stay on sync.
"""

import numpy as np

B, H, P, D = 4, 16, 1024, 128
N_CORES = 8
G = (B * H) // N_CORES  # heads per core = 8
PT = P // 128  # 8

_cached = {}


def _build_module():
    import concourse.bass as bass
    import concourse.tile as tile
    from concourse import bacc, mybir

    f32 = mybir.dt.float32
    bf16 = mybir.dt.bfloat16
    AF = mybir.ActivationFunctionType
    ALU = mybir.AluOpType

    nc = bacc.Bacc("TRN2", target_bir_lowering=False, debug=False)
    q_d = nc.dram_tensor("q", [G, P, D], bf16, kind="ExternalInput")
    k_d = nc.dram_tensor("k", [G, P, D], bf16, kind="ExternalInput")
    scale_d = nc.dram_tensor("scale", [1], f32, kind="ExternalInput")
    out_d = nc.dram_tensor("out", [G, P, P], bf16, kind="ExternalOutput")

    with tile.TileContext(nc) as tc:
        with (
            tc.tile_pool(name="consts", bufs=1) as consts,
            tc.tile_pool(name="xt", bufs=3) as xtp,
            tc.tile_pool(name="th", bufs=2) as thp,
            tc.tile_pool(name="bt", bufs=3) as btp,
            tc.tile_pool(name="exp", bufs=6) as expp,
            tc.tile_pool(name="outs", bufs=4) as outp,
            tc.tile_pool(name="stats", bufs=10) as statp,
            tc.tile_pool(name="ps", bufs=2, space="PSUM") as psp,
        ):
            xts = {}

            def dma_in(g):
                """xbar-transposed loads: xt[:, 0] = qT, xt[:, 1] = kT."""
                xt = xtp.tile([128, 2, P], bf16, tag="xt", name=f"xt{g}")
                nc.sync.dma_start_transpose(out=xt[:, 0, :], in_=q_d[g])
                nc.sync.dma_start_transpose(out=xt[:, 1, :], in_=k_d[g])
                xts[g] = xt

            # scale first (tiny, gates the first exp), then the first
            # input transposes, all on the sync queue.
            scale_p0 = consts.tile([1, 1], f32)
            nc.sync.dma_start(out=scale_p0, in_=scale_d[:])
            dma_in(0)
            dma_in(1)

            scale_sb = consts.tile([128, 1], f32)
            nc.gpsimd.partition_broadcast(scale_sb, scale_p0)
            # bT = 2*silu => scores are 4x; fold the 1/4 into the exp scale
            scale_adj = consts.tile([128, 1], f32)
            nc.vector.tensor_scalar_mul(scale_adj, scale_sb, 0.25)

            # PE p-state warmup: garbage matmuls while the first input
            # DMAs are in flight (results never read).
            junk = consts.tile([128, 512], bf16)
            nc.vector.memset(junk, 0.0)
            for w in range(3):
                ps = psp.tile([128, 2, P], f32, tag="ps", name=f"warm{w}")
                for mm in range(2):
                    for h in range(2):
                        nc.tensor.matmul(
                            ps[:, mm, bass.ts(h, 512)],
                            junk[:, 0:128],
                            junk,
                            start=True,
                            stop=True,
                        )

            bts = {}

            def emit_silu(g, split=False):
                """bT = 2*silu(xT) = (tanh(xT/2)+1)*xT for q|k in one
                tanh (ACT) + one stt (DVE) pass, all contiguous.  split=True
                (head 0) runs tanh per half so ACT starts as soon as the q
                transpose lands."""
                xt = xts.pop(g)
                th = thp.tile([128, 2, P], bf16, tag="th", name=f"th{g}")
                bt = btp.tile([128, 2, P], bf16, tag="bt", name=f"bt{g}")
                if split:
                    for i in range(2):
                        nc.scalar.activation(
                            out=th[:, i, :], in_=xt[:, i, :],
                            func=AF.Tanh, scale=0.5)
                else:
                    nc.scalar.activation(out=th, in_=xt, func=AF.Tanh, scale=0.5)
                nc.vector.scalar_tensor_tensor(
                    out=bt, in0=th, scalar=1.0, in1=xt,
                    op0=ALU.add, op1=ALU.mult,
                )
                bts[g] = bt

            emit_silu(0, split=True)
            emit_silu(1)

            def mm_pair(ps, btq, btk, pair):
                """4 matmuls filling one [128, 2, P] PSUM tile with score
                m-tiles (2*pair, 2*pair+1).  512 moving elements is the ISA
                max per matmul (one PSUM bank)."""
                for mm in range(2):
                    for h in range(2):
                        nc.tensor.matmul(
                            ps[:, mm, bass.ts(h, 512)],
                            btq[:, bass.ts(pair * 2 + mm, 128)],
                            btk[:, bass.ts(h, 512)],
                            start=True,
                            stop=True,
                        )

            for g in range(G):
                bt = bts.pop(g)
                btq, btk = bt[:, 0, :], bt[:, 1, :]
                # out rows in natural order: row r*128+j -> partition j, slot r
                ov = out_d[g].rearrange("(r j) n -> j r n", j=128)
                last = g == G - 1

                for half in range(2):
                    og = outp.tile([128, 4, P], bf16, tag="out",
                                   name=f"out{g}_{half}")
                    for pp in range(2):
                        pair = half * 2 + pp
                        singles = pair == 3  # tiles 6-7: ACT accum sums
                        ps = psp.tile([128, 2, P], f32, tag="ps",
                                      name=f"ps{g}_{pair}")
                        mm_pair(ps, btq, btk, pair)
                        ex = expp.tile([128, 2, P], f32, tag="exp",
                                       name=f"exp{g}_{pair}")
                        sm = statp.tile([128, 2], f32, tag="sum",
                                        name=f"sum{g}_{pair}")
                        if singles:
                            for t in range(2):
                                nc.scalar.activation(
                                    out=ex[:, t, :], in_=ps[:, t, :],
                                    func=AF.Exp, scale=scale_adj,
                                    accum_out=sm[:, t:t + 1])
                        else:
                            nc.scalar.activation(
                                out=ex, in_=ps, func=AF.Exp, scale=scale_adj)
                            nc.vector.tensor_reduce(
                                out=sm, in_=ex,
                                axis=mybir.AxisListType.X, op=ALU.add)
                        for t in range(2):
                            if last and t == 1:
                                # drain: odd tiles on DVE so the final
                                # normalizes run Pool/DVE in parallel
                                nc.vector.reciprocal(
                                    sm[:, t:t + 1], sm[:, t:t + 1])
                                nc.vector.tensor_scalar_mul(
                                    og[:, pp * 2 + t, :], ex[:, t, :],
                                    sm[:, t:t + 1])
                            else:
                                nc.gpsimd.normalize_recip(
                                    og[:, pp * 2 + t, :], ex[:, t, :],
                                    sm[:, t:t + 1])
                        # Lookahead: inputs two heads out early in the head;
                        # silu for g+1 after this head's DVE reduces so the
                        # stt never head-of-line blocks them.
                        if pair == 0 and g + 2 < G:
                            dma_in(g + 2)
                        elif pair == 2 and 1 <= g < G - 1:
                            emit_silu(g + 1)
                        if last:
                            # drain: ship each tile-pair as it completes
                            nc.sync.dma_start(
                                out=ov[:, pair * 2:pair * 2 + 2, :],
                                in_=og[:, pp * 2:pp * 2 + 2, :])
                    if not last:
                        nc.sync.dma_start(
                            out=ov[:, half * 4:(half + 1) * 4, :], in_=og)

    nc.compile()
    return nc


def _get_nc():
    if "nc" not in _cached:
        _cached["nc"] = _build_module()
    return _cached["nc"]


def kernel(q, k, scale, _trace=False):
    import ml_dtypes
    from concourse.bass_utils import run_bass_kernel_spmd

    nc = _get_nc()
    qf = np.asarray(q, dtype=np.float32).reshape(B * H, P, D).astype(
        ml_dtypes.bfloat16)
    kf = np.asarray(k, dtype=np.float32).reshape(B * H, P, D).astype(
        ml_dtypes.bfloat16)
    sc = np.ascontiguousarray(np.asarray(scale, dtype=np.float32).reshape(1))
    in_maps = [
        {"q": qf[i * G:(i + 1) * G], "k": kf[i * G:(i + 1) * G], "scale": sc}
        for i in range(N_CORES)
    ]
    res = run_bass_kernel_spmd(
        nc, in_maps, core_ids=list(range(N_CORES)), trace=_trace
    )
    out = np.empty((B * H, P, P), dtype=np.float32)
    for i in range(N_CORES):
        out[i * G:(i + 1) * G] = res.results[i]["out"]
    if _trace:
        kernel.last_result = res
    return out.reshape(B, H, P, P)


# revision 28
# speedup vs baseline: 1.0484x; 1.0484x over previous
"""Trainium2 Bass kernel for patch attention:
    out = softmax(silu(q) @ silu(k)^T * scale, axis=-1)
with q,k: [B=4, H=16, P=1024, D=128] fp32, scale: [1] fp32.

Sharding: B*H = 64 heads split across 8 NeuronCores, 8 heads each.

v6 design.  bf16 on the wire both ways (host casts inputs to bf16, upcasts
the bf16 output back to fp32); qT/kT loaded via xbar dma_start_transpose
(no PE transposes).  Per-head engine budget (us), measured rates:

  ACT   exp 3 pairs [128,2048] + 2 singles w/ accum + tanh = 10.6  <- clock
  DVE   silu stt 2.2 + 3 pair row-sum reduces (1x) 6.9     =  9.1
  Pool  8x normalize_recip [128,1024] f32->bf16            =  8.6
  PE    16 matmul 128x128x512 bf16 (+8 ldweights)          ~  8
  DMA   0.5 MB in + 2 MB out at ~400 GB/s                  ~  6.3

Design notes, from HW traces + the cost model:
* Row sums are the scarce resource: every DVE reduction path measures 1x
  (~1.1ns/elem) — tensor_reduce, pool, AND tensor_scalar-with-accum_out
  (the cost model advertises 2x_2p for the latter but silicon runs 1x).
  So tiles 0-5 batch exp in [128,2048] pairs (amortizing ACT's ~350-cycle
  per-op overhead; accum_out there would mix the two tiles' sums) with
  one DVE pair-reduce each, and tiles 6-7 run unbatched exp WITH
  accum_out, making their sums nearly free on ACT (+180ns read_accum).
* exp pairs read 4-bank PSUM tiles; 2 buffers = all 8 banks.  The
  exp(p) -> matmul(p+2) -> exp(p+2) handoff is latency-tight; a burst of
  dummy matmuls during the DMA ramp warms the PE p-state (cold PE runs
  MM 128x128x512 at ~390ns vs ~216ns spec) to soften it.
* Strided/3D DVE access patterns cost ~2.6x, so everything DVE touches is
  contiguous; score m-tiles stay in natural row order and the output DMA
  writes 2 KB per (partition, row) chunk instead (256 descs/start).
* tanh shares the exp ACT table set (one ACT_TABLE_LOAD); head 0's tanh
  is split in half so ACT starts on qT while kT is still in flight.
  silu(g+1) is emitted after head g's reduces so its stt never
  head-of-line blocks DVE.  The scale load is queued on sync BEFORE the
  transposes (it gates the first exp).
* Normalization runs on Pool in steady state: concurrent DVE
  tensor_scalar + Pool normalize_recip contend on SBUF ports (a 0.6us DVE
  mult measured 5us when overlapped with Pool normalize).  Only the last
  head's odd tiles normalize on DVE, and its output DMAs are split per
  tile-pair, so the drain runs Pool/DVE in parallel.
* Issuing the k-half dma_start_transpose from nc.scalar races (local
  head 1 wrong on 7 of 8 cores) — all input DMAs stay on sync.
"""

import numpy as np

B, H, P, D = 4, 16, 1024, 128
N_CORES = 8
G = (B * H) // N_CORES  # heads per core = 8
PT = P // 128  # 8

_cached = {}


def _build_module():
    import concourse.bass as bass
    import concourse.tile as tile
    from concourse import bacc, mybir

    f32 = mybir.dt.float32
    bf16 = mybir.dt.bfloat16
    AF = mybir.ActivationFunctionType
    ALU = mybir.AluOpType

    nc = bacc.Bacc("TRN2", target_bir_lowering=False, debug=False)
    q_d = nc.dram_tensor("q", [G, P, D], bf16, kind="ExternalInput")
    k_d = nc.dram_tensor("k", [G, P, D], bf16, kind="ExternalInput")
    scale_d = nc.dram_tensor("scale", [1], f32, kind="ExternalInput")
    out_d = nc.dram_tensor("out", [G, P, P], bf16, kind="ExternalOutput")

    with tile.TileContext(nc) as tc:
        with (
            tc.tile_pool(name="consts", bufs=1) as consts,
            tc.tile_pool(name="xt", bufs=3) as xtp,
            tc.tile_pool(name="th", bufs=2) as thp,
            tc.tile_pool(name="bt", bufs=3) as btp,
            tc.tile_pool(name="exp", bufs=6) as expp,
            tc.tile_pool(name="outs", bufs=4) as outp,
            tc.tile_pool(name="stats", bufs=10) as statp,
            tc.tile_pool(name="ps", bufs=2, space="PSUM") as psp,
        ):
            xts = {}

            def dma_in(g):
                """xbar-transposed loads: xt[:, 0] = qT, xt[:, 1] = kT."""
                xt = xtp.tile([128, 2, P], bf16, tag="xt", name=f"xt{g}")
                nc.sync.dma_start_transpose(out=xt[:, 0, :], in_=q_d[g])
                nc.sync.dma_start_transpose(out=xt[:, 1, :], in_=k_d[g])
                xts[g] = xt

            # scale first (tiny, gates the first exp), then the first
            # input transposes, all on the sync queue.
            scale_p0 = consts.tile([1, 1], f32)
            nc.sync.dma_start(out=scale_p0, in_=scale_d[:])
            dma_in(0)
            dma_in(1)

            scale_sb = consts.tile([128, 1], f32)
            nc.gpsimd.partition_broadcast(scale_sb, scale_p0)
            # bT = 2*silu => scores are 4x; fold the 1/4 into the exp scale
            scale_adj = consts.tile([128, 1], f32)
            nc.vector.tensor_scalar_mul(scale_adj, scale_sb, 0.25)

            # PE p-state warmup: garbage matmuls while the first input
            # DMAs are in flight (results never read).
            junk = consts.tile([128, 512], bf16)
            nc.vector.memset(junk, 0.0)
            for w in range(3):
                ps = psp.tile([128, 2, P], f32, tag="ps", name=f"warm{w}")
                for mm in range(2):
                    for h in range(2):
                        nc.tensor.matmul(
                            ps[:, mm, bass.ts(h, 512)],
                            junk[:, 0:128],
                            junk,
                            start=True,
                            stop=True,
                        )

            bts = {}

            def emit_silu(g, split=False):
                """bT = 2*silu(xT) = (tanh(xT/2)+1)*xT for q|k in one
                tanh (ACT) + one stt (DVE) pass, all contiguous.  split=True
                (head 0) runs tanh per half so ACT starts as soon as the q
                transpose lands."""
                xt = xts.pop(g)
                th = thp.tile([128, 2, P], bf16, tag="th", name=f"th{g}")
                bt = btp.tile([128, 2, P], bf16, tag="bt", name=f"bt{g}")
                if split:
                    for i in range(2):
                        nc.scalar.activation(
                            out=th[:, i, :], in_=xt[:, i, :],
                            func=AF.Tanh, scale=0.5)
                else:
                    nc.scalar.activation(out=th, in_=xt, func=AF.Tanh, scale=0.5)
                nc.vector.scalar_tensor_tensor(
                    out=bt, in0=th, scalar=1.0, in1=xt,
                    op0=ALU.add, op1=ALU.mult,
                )
                bts[g] = bt

            emit_silu(0, split=True)

            def mm_pair(ps, btq, btk, pair):
                """4 matmuls filling one [128, 2, P] PSUM tile with score
                m-tiles (2*pair, 2*pair+1).  512 moving elements is the ISA
                max per matmul (one PSUM bank)."""
                for mm in range(2):
                    for h in range(2):
                        nc.tensor.matmul(
                            ps[:, mm, bass.ts(h, 512)],
                            btq[:, bass.ts(pair * 2 + mm, 128)],
                            btk[:, bass.ts(h, 512)],
                            start=True,
                            stop=True,
                        )

            for g in range(G):
                bt = bts.pop(g)
                btq, btk = bt[:, 0, :], bt[:, 1, :]
                # out rows in natural order: row r*128+j -> partition j, slot r
                ov = out_d[g].rearrange("(r j) n -> j r n", j=128)
                last = g == G - 1

                for half in range(2):
                    og = outp.tile([128, 4, P], bf16, tag="out",
                                   name=f"out{g}_{half}")
                    for pp in range(2):
                        pair = half * 2 + pp
                        singles = pair == 3  # tiles 6-7: ACT accum sums
                        ps = psp.tile([128, 2, P], f32, tag="ps",
                                      name=f"ps{g}_{pair}")
                        mm_pair(ps, btq, btk, pair)
                        ex = expp.tile([128, 2, P], f32, tag="exp",
                                       name=f"exp{g}_{pair}")
                        sm = statp.tile([128, 2], f32, tag="sum",
                                        name=f"sum{g}_{pair}")
                        if singles:
                            for t in range(2):
                                nc.scalar.activation(
                                    out=ex[:, t, :], in_=ps[:, t, :],
                                    func=AF.Exp, scale=scale_adj,
                                    accum_out=sm[:, t:t + 1])
                        else:
                            nc.scalar.activation(
                                out=ex, in_=ps, func=AF.Exp, scale=scale_adj)
                            nc.vector.tensor_reduce(
                                out=sm, in_=ex,
                                axis=mybir.AxisListType.X, op=ALU.add)
                        for t in range(2):
                            if last and t == 1:
                                # drain: odd tiles on DVE so the final
                                # normalizes run Pool/DVE in parallel
                                nc.vector.reciprocal(
                                    sm[:, t:t + 1], sm[:, t:t + 1])
                                nc.vector.tensor_scalar_mul(
                                    og[:, pp * 2 + t, :], ex[:, t, :],
                                    sm[:, t:t + 1])
                            else:
                                nc.gpsimd.normalize_recip(
                                    og[:, pp * 2 + t, :], ex[:, t, :],
                                    sm[:, t:t + 1])
                        # Lookahead: inputs two heads out early in the head;
                        # silu for g+1 after this head's DVE reduces so the
                        # stt never head-of-line blocks them.
                        if pair == 0 and g + 2 < G:
                            dma_in(g + 2)
                        elif pair == 2 and g + 1 < G:
                            emit_silu(g + 1)
                        if last:
                            # drain: ship each tile-pair as it completes
                            nc.sync.dma_start(
                                out=ov[:, pair * 2:pair * 2 + 2, :],
                                in_=og[:, pp * 2:pp * 2 + 2, :])
                    if not last:
                        nc.sync.dma_start(
                            out=ov[:, half * 4:(half + 1) * 4, :], in_=og)

    nc.compile()
    return nc


def _get_nc():
    if "nc" not in _cached:
        _cached["nc"] = _build_module()
    return _cached["nc"]


def kernel(q, k, scale, _trace=False):
    import ml_dtypes
    from concourse.bass_utils import run_bass_kernel_spmd

    nc = _get_nc()
    qf = np.asarray(q, dtype=np.float32).reshape(B * H, P, D).astype(
        ml_dtypes.bfloat16)
    kf = np.asarray(k, dtype=np.float32).reshape(B * H, P, D).astype(
        ml_dtypes.bfloat16)
    sc = np.ascontiguousarray(np.asarray(scale, dtype=np.float32).reshape(1))
    in_maps = [
        {"q": qf[i * G:(i + 1) * G], "k": kf[i * G:(i + 1) * G], "scale": sc}
        for i in range(N_CORES)
    ]
    res = run_bass_kernel_spmd(
        nc, in_maps, core_ids=list(range(N_CORES)), trace=_trace
    )
    out = np.empty((B * H, P, P), dtype=np.float32)
    for i in range(N_CORES):
        out[i * G:(i + 1) * G] = res.results[i]["out"]
    if _trace:
        kernel.last_result = res
    return out.reshape(B, H, P, P)
